# revision 1
# baseline (speedup 1.0000x reference)
"""GAT (3-layer, 4-head) on 8 Trainium2 NeuronCores.

Sharding: nodes padded to 100352 = 8 * 98 * 128; core c owns the contiguous
dst-node range [c*12544, (c+1)*12544) and its incoming-edge CSR slice.
Per layer: local dense phase (feat = h @ W, attention scores el/er) writes a
packed [feat | el] row table slice; AllGather shares the full table; each
core then indirect-DMA-gathers its edges' source rows and does edge-softmax +
weighted aggregation on-chip. h stays SBUF-resident between layers.
"""
import os
import sys

sys.path.insert(0, "/opt/trn_rl_repo")

import numpy as np

P = 128
NCORES = 8
N = 100000
DEG = 16
HEADS = 4
HID = 64
IN_DIM = 256
NCLS = 41
NEG = 0.2

TPC = 98                  # dst tiles per core
NLOC = TPC * P            # 12544
NPAD = NCORES * NLOC      # 100352
D1 = HEADS * HID          # 256
D2 = HEADS * NCLS         # 164
ROW1 = D1 + 16            # packed row: 256 feat + 4 el + pad (544B, 32B-aligned)
ROW2 = D2 + 12            # 164 feat + 4 el + pad (352B, 32B-aligned)


def _pack_a(al, ar, fdim, hdim):
    """Block-diagonal [fdim, 8] selector: col h = al[h] in rows h*hdim..,
    col 4+h = ar[h]."""
    a = np.zeros((fdim, 8), np.float32)
    al = np.asarray(al, np.float32)
    ar = np.asarray(ar, np.float32)
    for h in range(HEADS):
        a[h * hdim:(h + 1) * hdim, h] = al[h]
        a[h * hdim:(h + 1) * hdim, 4 + h] = ar[h]
    return a


def build_program():
    import concourse.bass as bass
    import concourse.bacc as bacc
    import concourse.mybir as mybir
    import concourse.tile as tile
    from concourse.masks import make_identity

    f32 = mybir.dt.float32
    bf16 = mybir.dt.bfloat16
    nc = bacc.Bacc("TRN2", target_bir_lowering=False, debug=False,
                   num_devices=NCORES)

    hT0 = nc.declare_dram_parameter("hT0", [TPC * IN_DIM, P], f32, isOutput=False)
    idx_in = nc.declare_dram_parameter("idx", [P, TPC * DEG], mybir.dt.int32,
                                       isOutput=False)
    W0 = nc.declare_dram_parameter("W0", [IN_DIM, D1], f32, isOutput=False)
    W1 = nc.declare_dram_parameter("W1", [D1, D1], f32, isOutput=False)
    W2 = nc.declare_dram_parameter("W2", [D1, D2], f32, isOutput=False)
    A0 = nc.declare_dram_parameter("A0", [D1, 8], f32, isOutput=False)
    A1 = nc.declare_dram_parameter("A1", [D1, 8], f32, isOutput=False)
    A2 = nc.declare_dram_parameter("A2", [D2, 8], f32, isOutput=False)
    out_ext = nc.declare_dram_parameter("out", [NLOC, NCLS], f32, isOutput=True)

    Ws = [W0, W1, W2]
    As = [A0, A1, A2]
    DL = [D1, D1, D2]         # output feat dim per layer
    ROWL = [ROW1, ROW1, ROW2]
    GROW = DEG * ROW1         # G tile width, max over layers
    MSGW = DEG * D1           # msg width, max over layers

    with tile.TileContext(nc) as tc:
        with (
            tc.tile_pool(name="const", bufs=1) as cp,
            tc.tile_pool(name="resid", bufs=1) as rp,
            tc.tile_pool(name="wk", bufs=3) as wk,
            tc.tile_pool(name="seq", bufs=1) as sq,
            tc.tile_pool(name="gat", bufs=5) as gp,
            tc.tile_pool(name="psp", bufs=2, space="PSUM") as psp,
            tc.tile_pool(name="dram", bufs=1, space="DRAM") as dram,
        ):
            ident = cp.tile([P, P], f32)
            make_identity(nc, ident[:])

            # weights resident in SBUF
            wsb = []   # wsb[l][ic] : [128, DL[l]]
            asb = []   # asb[l][ic] : ([128, 8], chunk)
            for l in range(3):
                wl, al = [], []
                for ic in range(2):
                    w = cp.tile([P, DL[l]], f32, name=f"w{l}_{ic}")
                    nc.sync.dma_start(out=w[:], in_=Ws[l][ic * P:(ic + 1) * P, :])
                    wl.append(w)
                nch = (DL[l] + P - 1) // P
                for ic in range(nch):
                    lo = ic * P
                    hi = min(DL[l], lo + P)
                    a = cp.tile([P, 8], f32, name=f"a{l}_{ic}")
                    nc.sync.dma_start(out=a[:hi - lo, :], in_=As[l][lo:hi, :])
                    al.append((a, hi - lo))
                wsb.append(wl)
                asb.append(al)

            # resident buffers
            h_res = rp.tile([P, TPC * D1], f32)          # 12.8 MB
            er_res = rp.tile([P, TPC * 4], bf16)
            idxs = rp.tile([P, TPC * DEG], mybir.dt.int32)
            nc.sync.dma_start(out=idxs[:], in_=idx_in[:])

            # DRAM tables (bf16 packed rows)
            ag_in = [dram.tile([NLOC, ROWL[l]], bf16, name=f"agin{l}")
                     for l in range(3)]
            table = [dram.tile([NPAD, ROWL[l]], bf16, addr_space="Shared",
                               name=f"table{l}")
                     for l in range(3)]

            for l in range(3):
                DO = DL[l]
                ROW = ROWL[l]
                hd = DO // HEADS
                # ---------------- dense phase ----------------
                for t in range(TPC):
                    hT = []
                    if l == 0:
                        for ic in range(2):
                            ht = wk.tile([P, P], f32, tag="ht", name=f"ht{l}_{t}_{ic}")
                            nc.sync.dma_start(
                                out=ht[:],
                                in_=hT0[t * IN_DIM + ic * P:
                                        t * IN_DIM + (ic + 1) * P, :])
                            hT.append(ht)
                    else:
                        for ic in range(2):
                            tp = psp.tile([P, P], f32, tag="tp", bufs=2,
                                          name=f"tp{l}_{t}_{ic}")
                            nc.tensor.transpose(
                                tp[:],
                                h_res[:, t * D1 + ic * P: t * D1 + (ic + 1) * P],
                                ident[:])
                            ht = wk.tile([P, P], f32, tag="ht", name=f"ht{l}_{t}_{ic}")
                            nc.scalar.copy(ht[:], tp[:])
                            hT.append(ht)

                    packed = wk.tile([P, ROW1], bf16, tag="pk", name=f"pk{l}_{t}")
                    noc = (DO + P - 1) // P
                    fTs = []
                    for oc in range(noc):
                        lo = oc * P
                        hi = min(DO, lo + P)
                        w = hi - lo
                        fp = psp.tile([P, P], f32, tag="fp", bufs=2,
                                      name=f"fp{l}_{t}_{oc}")
                        nc.tensor.matmul(fp[:w, :], wsb[l][0][:, lo:hi], hT[0][:],
                                         start=True, stop=False)
                        nc.tensor.matmul(fp[:w, :], wsb[l][1][:, lo:hi], hT[1][:],
                                         start=False, stop=True)
                        fT = wk.tile([P, P], f32, tag="fT", name=f"fT{l}_{t}_{oc}")
                        cpy = nc.vector.tensor_copy if l == 0 else nc.scalar.copy
                        cpy(fT[:w, :], fp[:w, :])
                        fTs.append((fT, w))
                        bk = psp.tile([P, P], f32, tag="bk", bufs=2,
                                      name=f"bk{l}_{t}_{oc}")
                        nc.tensor.transpose(bk[:, :w], fT[:w, :], ident[:w, :w])
                        cpy(packed[:, lo:hi], bk[:, :w])

                    # el/er: contract over DO
                    ep = psp.tile([8, P], f32, tag="ep", bufs=1, name=f"ep{l}_{t}")
                    nf = len(fTs)
                    for ic, (fT, w) in enumerate(fTs):
                        nc.tensor.matmul(ep[:, :], asb[l][ic][0][:w, :], fT[:w, :],
                                         start=(ic == 0), stop=(ic == nf - 1))
                    es = wk.tile([8, P], f32, tag="es", name=f"es{l}_{t}")
                    nc.vector.tensor_copy(es[:], ep[:])
                    et = psp.tile([P, 8], f32, tag="et", bufs=1, name=f"et{l}_{t}")
                    nc.tensor.transpose(et[:, :], es[:, :], ident[:8, :8])
                    nc.vector.tensor_copy(packed[:, DO:DO + 4], et[:, 0:4])
                    nc.vector.tensor_copy(er_res[:, t * 4:(t + 1) * 4], et[:, 4:8])

                    nc.sync.dma_start(out=ag_in[l][t * P:(t + 1) * P, :],
                                      in_=packed[:, :ROW])

                # ---------------- share ----------------
                nc.gpsimd.collective_compute(
                    "AllGather",
                    mybir.AluOpType.bypass,
                    replica_groups=[list(range(NCORES))],
                    ins=[ag_in[l][:]],
                    outs=[table[l][:]],
                )

                # ---------------- gather + aggregate ----------------
                for t in range(TPC):
                    G = gp.tile([P, GROW], bf16, tag="G", name=f"G{l}_{t}")
                    for k in range(DEG):
                        nc.gpsimd.indirect_dma_start(
                            out=G[:, k * ROW:(k + 1) * ROW],
                            out_offset=None,
                            in_=table[l][:],
                            in_offset=bass.IndirectOffsetOnAxis(
                                ap=idxs[:, t * DEG + k:t * DEG + k + 1], axis=0),
                        )
                    Gv = G[:, :DEG * ROW].rearrange("p (k r) -> p k r", k=DEG)
                    # e = lrelu(el_src + er_dst) ; layout [p, h(4), k(16)]
                    e = wk.tile([P, 64], f32, tag="e", name=f"e{l}_{t}")
                    el_view = Gv[:, :, DO:DO + 4].rearrange("p k h -> p h k")
                    er_b = er_res[:, t * 4:(t + 1) * 4].to_broadcast([P, 4, DEG])
                    nc.vector.tensor_tensor(
                        out=e[:].rearrange("p (h k) -> p h k", h=4),
                        in0=el_view, in1=er_b, op=mybir.AluOpType.add)
                    esc = wk.tile([P, 64], f32, tag="esc", name=f"esc{l}_{t}")
                    nc.vector.tensor_scalar_mul(esc[:], e[:], NEG)
                    nc.vector.tensor_max(e[:], e[:], esc[:])
                    ex = wk.tile([P, 64], bf16, tag="ex", name=f"ex{l}_{t}")
                    nc.scalar.activation(ex[:], e[:],
                                         mybir.ActivationFunctionType.Exp)
                    den = wk.tile([P, 4], f32, tag="den", name=f"den{l}_{t}")
                    nc.vector.tensor_reduce(
                        out=den[:], in_=ex[:].rearrange("p (h k) -> p h k", h=4),
                        axis=mybir.AxisListType.X, op=mybir.AluOpType.add)
                    rden = wk.tile([P, 4], f32, tag="rden", name=f"rden{l}_{t}")
                    nc.vector.reciprocal(rden[:], den[:])
                    if l == 2:
                        nc.vector.tensor_scalar_mul(rden[:], rden[:], 1.0 / HEADS)
                    # alpha = ex * rden (fold the softmax denominator in now,
                    # so the k-reduce output is final)
                    alp = wk.tile([P, 64], bf16, tag="alp", name=f"alp{l}_{t}")
                    nc.vector.tensor_tensor(
                        out=alp[:].rearrange("p (h k) -> p h k", h=4),
                        in0=ex[:].rearrange("p (h k) -> p h k", h=4),
                        in1=rden[:].to_broadcast([P, 4, DEG]),
                        op=mybir.AluOpType.mult)

                    # msg = G_feat * ex  (bcast over d)
                    msg = sq.tile([P, MSGW], bf16, tag="msg", name=f"msg{l}_{t}")
                    gfeat = Gv[:, :, 0:DO].rearrange("p k (h d) -> p k h d",
                                                     h=HEADS)
                    exb = alp[:].rearrange("p (h k) -> p k h", h=4) \
                               .to_broadcast([P, DEG, 4, hd])
                    nc.vector.tensor_tensor(
                        out=msg[:, :DEG * DO].rearrange(
                            "p (k h d) -> p k h d", k=DEG, h=HEADS),
                        in0=gfeat, in1=exb, op=mybir.AluOpType.mult)

                    # tree-reduce over k
                    cur = msg
                    width = DEG * DO
                    lvl = 0
                    while width > DO:
                        width //= 2
                        nxt = sq.tile([P, width], bf16, tag=f"s{lvl}",
                                      name=f"s{l}_{t}_{lvl}")
                        nc.vector.tensor_add(nxt[:], cur[:, 0:width],
                                             cur[:, width:2 * width])
                        cur = nxt
                        lvl += 1

                    if l < 2:
                        nc.vector.tensor_copy(
                            h_res[:, t * D1:(t + 1) * D1], cur[:, :DO])
                    else:
                        lg = wk.tile([P, NCLS], f32, tag="lg", name=f"lg{l}_{t}")
                        nc.vector.tensor_reduce(
                            out=lg[:],
                            in_=cur[:, :DO].rearrange("p (h c) -> p c h", h=HEADS),
                            axis=mybir.AxisListType.X, op=mybir.AluOpType.add)
                        nc.sync.dma_start(out=out_ext[t * P:(t + 1) * P, :],
                                          in_=lg[:])

    nc.compile()
    return nc


def prep_inputs(row_ptr, col_ind, inputs, W0, al0, ar0, W1, al1, ar1,
                W2, al2, ar2):
    col = np.asarray(col_ind, np.int32).reshape(N, DEG)
    col_pad = np.zeros((NPAD, DEG), np.int32)
    col_pad[:N] = col
    x = np.asarray(inputs, np.float32)
    x_pad = np.zeros((NPAD, IN_DIM), np.float32)
    x_pad[:N] = x

    a0 = _pack_a(al0, ar0, D1, HID)
    a1 = _pack_a(al1, ar1, D1, HID)
    a2 = _pack_a(al2, ar2, D2, NCLS)

    in_maps = []
    for c in range(NCORES):
        lo = c * NLOC
        xT = x_pad[lo:lo + NLOC].T                               # [256, NLOC]
        hT0c = np.ascontiguousarray(
            xT.reshape(IN_DIM, TPC, P).transpose(1, 0, 2)
              .reshape(TPC * IN_DIM, P))                           # tile-major
        ic = col_pad[lo:lo + NLOC]                              # [NLOC, 16]
        ia = ic.reshape(TPC, P, DEG).transpose(1, 0, 2).reshape(P, TPC * DEG)
        in_maps.append({
            "hT0": hT0c,
            "idx": np.ascontiguousarray(ia),
            "W0": np.asarray(W0, np.float32),
            "W1": np.asarray(W1, np.float32),
            "W2": np.asarray(W2, np.float32),
            "A0": a0, "A1": a1, "A2": a2,
        })
    return in_maps


_NC_CACHE = {}


def kernel(**inputs):
    from concourse.bass_utils import run_bass_kernel_spmd

    if "nc" not in _NC_CACHE:
        _NC_CACHE["nc"] = build_program()
    nc = _NC_CACHE["nc"]

    in_maps = prep_inputs(**inputs)

    trace = bool(int(os.environ.get("BASS_GAT_TRACE", "0")))
    res = run_bass_kernel_spmd(nc, in_maps, list(range(NCORES)), trace=trace)
    _NC_CACHE["last_exec_ns"] = res.exec_time_ns

    out = np.concatenate([res.results[c]["out"] for c in range(NCORES)], axis=0)
    return np.ascontiguousarray(out[:N])



# revision 5
# speedup vs baseline: 1.3031x; 1.3031x over previous
"""GAT (3-layer, 4-head) on 8 Trainium2 NeuronCores — v3.

Sharding: nodes padded to 100352 = 8 * 98 * 128; core c owns the contiguous
dst-node range [c*12544, (c+1)*12544) and its incoming-edge CSR slice.

Structure (software-pipelined):
  dense_0(all tiles) ; chunked AG_0
  for l in 0..2:
      for t in tiles:  gather_l(t) ; agg_l(t) ; dense_{l+1}(t)
      chunked AG_{l+1}  (chunks fire as their tiles complete)
Layer l+1's dense work rides the same engine queues interleaved per-tile with
layer l's aggregation, so PE/Scalar run during the gather phase instead of
serializing at the layer boundary.

Table is chunk-major: AllGather runs per 14-tile chunk (7 chunks), so all but
the last chunk overlap the dense phase. The host remaps edge src -> table row
(chunk-major), so the kernel's gather is unchanged.

Features interleaved (d, h) so the alpha-broadcast multiply runs in DVE 2x
mode (innermost step 1 on all operands). Weights permuted on the host.

Gather: batched indirect DMA with an iterating indirection dim — src gets a
stride-0 leading dim [KB, NPAD, ROW], dst [128, KB, ROWP] with padded row
stride so the AP doesn't merge; each indirection visit consumes a fresh
128-wide index set (one column of idx [128, KB]).
"""
import os
import sys

sys.path.insert(0, "/opt/trn_rl_repo")

import numpy as np

P = 128
NCORES = 8
N = 100000
DEG = 16
HEADS = 4
HID = 64
IN_DIM = 256
NCLS = 41
NEG = 0.2

TPC = 98                  # dst tiles per core
NLOC = TPC * P            # 12544
NPAD = NCORES * NLOC      # 100352
D1 = HEADS * HID          # 256
D2 = HEADS * NCLS         # 164
ROW1 = D1 + 16            # packed row: 256 feat + 4 el + pad (544B)
ROW2 = 256                # 164 feat + 4 el + pad to 512B rows
ROWP1 = ROW1 + 8          # padded dst row stride (defeats AP merging)
ROWP2 = ROW2 + 8
KB = 8                    # idx columns per gather instruction (2 per tile)

# Shared DRAM tensors only admit a single writing instruction, so the
# AllGather cannot be chunked; one collective per layer.
NCH = 1
TCH = TPC // NCH
CHROWS = TCH * P


def _perm(hdim):
    """new feature j = d*HEADS + h  <-  original index h*hdim + d."""
    p = np.empty(HEADS * hdim, np.int64)
    for d in range(hdim):
        for h in range(HEADS):
            p[d * HEADS + h] = h * hdim + d
    return p


PERM1 = _perm(HID)        # 256
PERM2 = _perm(NCLS)       # 164


def _pack_a(al, ar, fdim, hdim):
    """Block-diagonal [fdim, 8] selector: col h = al[h] in rows h*hdim..,
    col 4+h = ar[h]. Rows in ORIGINAL feature order (permute after)."""
    a = np.zeros((fdim, 8), np.float32)
    al = np.asarray(al, np.float32)
    ar = np.asarray(ar, np.float32)
    for h in range(HEADS):
        a[h * hdim:(h + 1) * hdim, h] = al[h]
        a[h * hdim:(h + 1) * hdim, 4 + h] = ar[h]
    return a


def build_program():
    import concourse.bass as bass
    import concourse.bacc as bacc
    import concourse.mybir as mybir
    import concourse.tile as tile
    from concourse.masks import make_identity

    f32 = mybir.dt.float32
    bf16 = mybir.dt.bfloat16
    nc = bacc.Bacc("TRN2", target_bir_lowering=False, debug=False,
                   num_devices=NCORES)

    hT0 = nc.declare_dram_parameter("hT0", [TPC * IN_DIM, P], f32, isOutput=False)
    idx_in = nc.declare_dram_parameter("idx", [P, TPC * DEG], mybir.dt.int32,
                                       isOutput=False)
    W0 = nc.declare_dram_parameter("W0", [IN_DIM, D1], f32, isOutput=False)
    W1 = nc.declare_dram_parameter("W1", [D1, D1], f32, isOutput=False)
    W2 = nc.declare_dram_parameter("W2", [D1, D2], f32, isOutput=False)
    A0 = nc.declare_dram_parameter("A0", [D1, 8], f32, isOutput=False)
    A1 = nc.declare_dram_parameter("A1", [D1, 8], f32, isOutput=False)
    A2 = nc.declare_dram_parameter("A2", [D2, 8], f32, isOutput=False)
    out_ext = nc.declare_dram_parameter("out", [NLOC, NCLS], f32, isOutput=True)

    Ws = [W0, W1, W2]
    As = [A0, A1, A2]
    DL = [D1, D1, D2]
    ROWL = [ROW1, ROW1, ROW2]
    ROWPL = [ROWP1, ROWP1, ROWP2]
    GROW = DEG * ROWP1        # G tile width (elems), max over layers
    MSGW = DEG * D1

    with tile.TileContext(nc) as tc:
        with (
            tc.tile_pool(name="const", bufs=1) as cp,
            tc.tile_pool(name="resid", bufs=1) as rp,
            tc.tile_pool(name="wk", bufs=3) as wk,
            tc.tile_pool(name="seq", bufs=1) as sq,
            tc.tile_pool(name="gat", bufs=3) as gp,
            tc.tile_pool(name="psp", bufs=2, space="PSUM") as psp,
            tc.tile_pool(name="dram", bufs=1, space="DRAM") as dram,
        ):
            ident = cp.tile([P, P], f32)
            make_identity(nc, ident[:])

            wsb = []
            asb = []
            for l in range(3):
                wl, al = [], []
                for ic in range(2):
                    w = cp.tile([P, DL[l]], f32, name=f"w{l}_{ic}")
                    nc.sync.dma_start(out=w[:], in_=Ws[l][ic * P:(ic + 1) * P, :])
                    wl.append(w)
                nch = (DL[l] + P - 1) // P
                for ic in range(nch):
                    lo = ic * P
                    hi = min(DL[l], lo + P)
                    a = cp.tile([P, 8], f32, name=f"a{l}_{ic}")
                    nc.sync.dma_start(out=a[:hi - lo, :], in_=As[l][lo:hi, :])
                    al.append((a, hi - lo))
                wsb.append(wl)
                asb.append(al)

            h_res = rp.tile([P, TPC * D1], f32)          # 12.8 MB
            er_res = rp.tile([P, TPC * 4], bf16)
            idxs = rp.tile([P, TPC * DEG], mybir.dt.int32)
            nc.sync.dma_start(out=idxs[:], in_=idx_in[:])

            ag_in = [dram.tile([NLOC, ROWL[l]], bf16, name=f"agin{l}")
                     for l in range(3)]
            table = [dram.tile([NPAD, ROWL[l]], bf16, addr_space="Shared",
                               name=f"table{l}")
                     for l in range(3)]

            def dense_tile(l, t):
                """feat = h @ W for dst tile t of layer l; writes packed row
                slice to ag_in[l] and er to er_res."""
                DO = DL[l]
                ROW = ROWL[l]
                hT = []
                if l == 0:
                    for ic in range(2):
                        ht = wk.tile([P, P], f32, tag="ht", name=f"ht{l}_{t}_{ic}")
                        nc.sync.dma_start(
                            out=ht[:],
                            in_=hT0[t * IN_DIM + ic * P:
                                    t * IN_DIM + (ic + 1) * P, :])
                        hT.append(ht)
                else:
                    for ic in range(2):
                        tp = psp.tile([P, P], f32, tag="tp", bufs=2,
                                      name=f"tp{l}_{t}_{ic}")
                        nc.tensor.transpose(
                            tp[:],
                            h_res[:, t * D1 + ic * P: t * D1 + (ic + 1) * P],
                            ident[:])
                        ht = wk.tile([P, P], f32, tag="ht", name=f"ht{l}_{t}_{ic}")
                        nc.scalar.copy(ht[:], tp[:])
                        hT.append(ht)

                packed = wk.tile([P, ROW1], bf16, tag="pk", name=f"pk{l}_{t}")
                noc = (DO + P - 1) // P
                fTs = []
                for oc in range(noc):
                    lo = oc * P
                    hi = min(DO, lo + P)
                    w = hi - lo
                    fp = psp.tile([P, P], f32, tag="fp", bufs=2,
                                  name=f"fp{l}_{t}_{oc}")
                    nc.tensor.matmul(fp[:w, :], wsb[l][0][:, lo:hi], hT[0][:],
                                     start=True, stop=False)
                    nc.tensor.matmul(fp[:w, :], wsb[l][1][:, lo:hi], hT[1][:],
                                     start=False, stop=True)
                    fT = wk.tile([P, P], f32, tag="fT", name=f"fT{l}_{t}_{oc}")
                    nc.scalar.copy(fT[:w, :], fp[:w, :])
                    fTs.append((fT, w))
                    bk = psp.tile([P, P], f32, tag="bk", bufs=2,
                                  name=f"bk{l}_{t}_{oc}")
                    nc.tensor.transpose(bk[:, :w], fT[:w, :], ident[:w, :w])
                    nc.scalar.copy(packed[:, lo:hi], bk[:, :w])

                # el/er transposed directly: et2[n, 8] = fT^T @ A-chunk
                et2 = psp.tile([P, 8], f32, tag="et2", bufs=2, name=f"et2{l}_{t}")
                nf = len(fTs)
                for ic, (fT, w) in enumerate(fTs):
                    nc.tensor.matmul(et2[:, :], fTs[ic][0][:fTs[ic][1], :],
                                     asb[l][ic][0][:fTs[ic][1], :],
                                     start=(ic == 0), stop=(ic == nf - 1))
                nc.vector.tensor_copy(packed[:, DO:DO + 4], et2[:, 0:4])
                nc.vector.tensor_copy(er_res[:, t * 4:(t + 1) * 4], et2[:, 4:8])

                nc.sync.dma_start(out=ag_in[l][t * P:(t + 1) * P, :],
                                  in_=packed[:, :ROW])

            def share_chunk(l, j):
                """AllGather chunk j of layer l's table (rows j*CHROWS ..)."""
                r0 = j * CHROWS
                r1 = r0 + CHROWS
                nc.gpsimd.collective_compute(
                    "AllGather",
                    mybir.AluOpType.bypass,
                    replica_groups=[list(range(NCORES))],
                    ins=[ag_in[l][r0:r1]],
                    outs=[table[l][j * NCORES * CHROWS:(j + 1) * NCORES * CHROWS]],
                )

            def gather_agg_tile(l, t):
                DO = DL[l]
                ROW = ROWL[l]
                ROWP = ROWPL[l]
                hd = DO // HEADS
                G = gp.tile([P, GROW], bf16, tag="G", name=f"G{l}_{t}")
                Gv = G[:, :DEG * ROWP].rearrange("p (k r) -> p k r", k=DEG)
                # per-edge-slot indirect DMA ([128,1] offsets): the only
                # form the indirect1d ucode supports (one desc per partition)
                for k in range(DEG):
                    nc.gpsimd.indirect_dma_start(
                        out=Gv[:, k, :ROW],
                        out_offset=None,
                        in_=table[l][:],
                        in_offset=bass.IndirectOffsetOnAxis(
                            ap=idxs[:, t * DEG + k:t * DEG + k + 1], axis=0),
                    )
                # e = lrelu(el_src + er_dst) in (k h) layout
                e = wk.tile([P, 64], f32, tag="e", name=f"e{l}_{t}")
                el_view = Gv[:, :, DO:DO + 4]
                er_b = er_res[:, t * 4:(t + 1) * 4] \
                    .unsqueeze(1).broadcast_to([P, DEG, 4])
                nc.vector.tensor_tensor(
                    out=e[:].rearrange("p (k h) -> p k h", h=4),
                    in0=el_view, in1=er_b, op=mybir.AluOpType.add)
                esc = wk.tile([P, 64], f32, tag="esc", name=f"esc{l}_{t}")
                nc.vector.tensor_scalar_mul(esc[:], e[:], NEG)
                nc.vector.tensor_max(e[:], e[:], esc[:])
                ex = wk.tile([P, 64], bf16, tag="ex", name=f"ex{l}_{t}")
                nc.scalar.activation(ex[:], e[:],
                                     mybir.ActivationFunctionType.Exp)
                den = wk.tile([P, 4], f32, tag="den", name=f"den{l}_{t}")
                nc.vector.tensor_reduce(
                    out=den[:], in_=ex[:].rearrange("p (k h) -> p h k", h=4),
                    axis=mybir.AxisListType.X, op=mybir.AluOpType.add)
                rden = wk.tile([P, 4], f32, tag="rden", name=f"rden{l}_{t}")
                nc.vector.reciprocal(rden[:], den[:])
                if l == 2:
                    nc.vector.tensor_scalar_mul(rden[:], rden[:], 1.0 / HEADS)
                alp = wk.tile([P, 64], bf16, tag="alp", name=f"alp{l}_{t}")
                nc.vector.tensor_tensor(
                    out=alp[:].rearrange("p (k h) -> p k h", h=4),
                    in0=ex[:].rearrange("p (k h) -> p k h", h=4),
                    in1=rden[:].unsqueeze(1).broadcast_to([P, DEG, 4]),
                    op=mybir.AluOpType.mult)

                # msg = G_feat * alpha (bcast over d; innermost h) -> 2x mode
                msg = sq.tile([P, MSGW], bf16, tag="msg", name=f"msg{l}_{t}")
                gfeat = Gv[:, :, 0:DO].rearrange("p k (d h) -> p k d h", h=HEADS)
                alp_b = alp[:].rearrange("p (k h) -> p k h", h=4) \
                    .unsqueeze(2).broadcast_to([P, DEG, hd, HEADS])
                nc.vector.tensor_tensor(
                    out=msg[:, :DEG * DO].rearrange(
                        "p (k d h) -> p k d h", k=DEG, h=HEADS),
                    in0=gfeat, in1=alp_b, op=mybir.AluOpType.mult)

                cur = msg
                width = DEG * DO
                lvl = 0
                while width > DO:
                    width //= 2
                    nxt = sq.tile([P, width], bf16, tag=f"s{lvl}",
                                  name=f"s{l}_{t}_{lvl}")
                    nc.vector.tensor_add(nxt[:], cur[:, 0:width],
                                         cur[:, width:2 * width])
                    cur = nxt
                    lvl += 1

                if l < 2:
                    nc.scalar.copy(h_res[:, t * D1:(t + 1) * D1], cur[:, :DO])
                else:
                    lg = wk.tile([P, NCLS], f32, tag="lg", name=f"lg{l}_{t}")
                    nc.vector.tensor_reduce(
                        out=lg[:],
                        in_=cur[:, :DO].rearrange("p (c h) -> p c h", h=HEADS),
                        axis=mybir.AxisListType.X, op=mybir.AluOpType.add)
                    nc.sync.dma_start(out=out_ext[t * P:(t + 1) * P, :],
                                      in_=lg[:])

            # ---------------- pipelined schedule ----------------
            for t in range(TPC):
                dense_tile(0, t)
                if (t + 1) % TCH == 0:
                    share_chunk(0, t // TCH)
            for l in range(3):
                for t in range(TPC):
                    gather_agg_tile(l, t)
                    if l < 2:
                        dense_tile(l + 1, t)
                        if (t + 1) % TCH == 0:
                            share_chunk(l + 1, t // TCH)

    nc.compile()
    return nc


def _row_map():
    """src global node g -> chunk-major table row."""
    g = np.arange(NPAD, dtype=np.int64)
    c = g // NLOC
    r = g % NLOC
    j = r // CHROWS
    return (j * (NCORES * CHROWS) + c * CHROWS + (r % CHROWS)).astype(np.int32)


def prep_inputs(row_ptr, col_ind, inputs, W0, al0, ar0, W1, al1, ar1,
                W2, al2, ar2):
    col = np.asarray(col_ind, np.int32).reshape(N, DEG)
    col_pad = np.zeros((NPAD, DEG), np.int32)
    col_pad[:N] = col
    rowmap = _row_map()
    col_pad = rowmap[col_pad]                               # chunk-major rows
    x = np.asarray(inputs, np.float32)
    x_pad = np.zeros((NPAD, IN_DIM), np.float32)
    x_pad[:N] = x

    W0n = np.asarray(W0, np.float32)[:, PERM1]
    W1n = np.asarray(W1, np.float32)[PERM1][:, PERM1]
    W2n = np.asarray(W2, np.float32)[PERM1][:, PERM2]
    a0 = _pack_a(al0, ar0, D1, HID)[PERM1]
    a1 = _pack_a(al1, ar1, D1, HID)[PERM1]
    a2 = _pack_a(al2, ar2, D2, NCLS)[PERM2]

    in_maps = []
    for c in range(NCORES):
        lo = c * NLOC
        xT = x_pad[lo:lo + NLOC].T
        hT0c = np.ascontiguousarray(
            xT.reshape(IN_DIM, TPC, P).transpose(1, 0, 2)
              .reshape(TPC * IN_DIM, P))
        ic = col_pad[lo:lo + NLOC]
        ia = ic.reshape(TPC, P, DEG).transpose(1, 0, 2).reshape(P, TPC * DEG)
        in_maps.append({
            "hT0": hT0c,
            "idx": np.ascontiguousarray(ia),
            "W0": W0n,
            "W1": W1n,
            "W2": W2n,
            "A0": a0, "A1": a1, "A2": a2,
        })
    return in_maps


_NC_CACHE = {}


def kernel(**inputs):
    from concourse.bass_utils import run_bass_kernel_spmd

    if "nc" not in _NC_CACHE:
        _NC_CACHE["nc"] = build_program()
    nc = _NC_CACHE["nc"]

    in_maps = prep_inputs(**inputs)

    trace = bool(int(os.environ.get("BASS_GAT_TRACE", "0")))
    res = run_bass_kernel_spmd(nc, in_maps, list(range(NCORES)), trace=trace)
    _NC_CACHE["last_exec_ns"] = res.exec_time_ns

    out = np.concatenate([res.results[c]["out"] for c in range(NCORES)], axis=0)
    return np.ascontiguousarray(out[:N])


# revision 6
# speedup vs baseline: 1.3089x; 1.0045x over previous
"""GAT (3-layer, 4-head) on 8 Trainium2 NeuronCores — v3.

Sharding: nodes padded to 100352 = 8 * 98 * 128; core c owns the contiguous
dst-node range [c*12544, (c+1)*12544) and its incoming-edge CSR slice.

Structure (software-pipelined):
  dense_0(all tiles) ; chunked AG_0
  for l in 0..2:
      for t in tiles:  gather_l(t) ; agg_l(t) ; dense_{l+1}(t)
      chunked AG_{l+1}  (chunks fire as their tiles complete)
Layer l+1's dense work rides the same engine queues interleaved per-tile with
layer l's aggregation, so PE/Scalar run during the gather phase instead of
serializing at the layer boundary.

Table is chunk-major: AllGather runs per 14-tile chunk (7 chunks), so all but
the last chunk overlap the dense phase. The host remaps edge src -> table row
(chunk-major), so the kernel's gather is unchanged.

Features interleaved (d, h) so the alpha-broadcast multiply runs in DVE 2x
mode (innermost step 1 on all operands). Weights permuted on the host.

Gather: batched indirect DMA with an iterating indirection dim — src gets a
stride-0 leading dim [KB, NPAD, ROW], dst [128, KB, ROWP] with padded row
stride so the AP doesn't merge; each indirection visit consumes a fresh
128-wide index set (one column of idx [128, KB]).
"""
import os
import sys

sys.path.insert(0, "/opt/trn_rl_repo")

import numpy as np

P = 128
NCORES = 8
N = 100000
DEG = 16
HEADS = 4
HID = 64
IN_DIM = 256
NCLS = 41
NEG = 0.2

TPC = 98                  # dst tiles per core
NLOC = TPC * P            # 12544
NPAD = NCORES * NLOC      # 100352
D1 = HEADS * HID          # 256
D2 = HEADS * NCLS         # 164
ROW1 = D1 + 16            # packed row: 256 feat + 4 el + pad (544B)
ROW2 = 256                # 164 feat + 4 el + pad to 512B rows
ROWP1 = ROW1 + 8          # padded dst row stride (defeats AP merging)
ROWP2 = ROW2 + 8
KB = 8                    # idx columns per gather instruction (2 per tile)

# Shared DRAM tensors only admit a single writing instruction, so the
# AllGather cannot be chunked; one collective per layer.
NCH = 1
TCH = TPC // NCH
CHROWS = TCH * P


def _perm(hdim):
    """new feature j = d*HEADS + h  <-  original index h*hdim + d."""
    p = np.empty(HEADS * hdim, np.int64)
    for d in range(hdim):
        for h in range(HEADS):
            p[d * HEADS + h] = h * hdim + d
    return p


PERM1 = _perm(HID)        # 256
PERM2 = _perm(NCLS)       # 164


def _pack_a(al, ar, fdim, hdim):
    """Block-diagonal [fdim, 8] selector: col h = al[h] in rows h*hdim..,
    col 4+h = ar[h]. Rows in ORIGINAL feature order (permute after)."""
    a = np.zeros((fdim, 8), np.float32)
    al = np.asarray(al, np.float32)
    ar = np.asarray(ar, np.float32)
    for h in range(HEADS):
        a[h * hdim:(h + 1) * hdim, h] = al[h]
        a[h * hdim:(h + 1) * hdim, 4 + h] = ar[h]
    return a


def build_program():
    import concourse.bass as bass
    import concourse.bacc as bacc
    import concourse.mybir as mybir
    import concourse.tile as tile
    from concourse.masks import make_identity

    f32 = mybir.dt.float32
    bf16 = mybir.dt.bfloat16
    # 32KB dynamic-DMA scratch -> 2048-desc SWDGE ring: 16 gather
    # instructions in flight instead of 8, so descriptor generation does not
    # stall on ring reclaim.
    nc = bacc.Bacc("TRN2", target_bir_lowering=False, debug=False,
                   num_devices=NCORES, dynamic_dma_scratch_size=32768)

    hT0 = nc.declare_dram_parameter("hT0", [TPC * IN_DIM, P], f32, isOutput=False)
    idx_in = nc.declare_dram_parameter("idx", [P, TPC * DEG], mybir.dt.int32,
                                       isOutput=False)
    W0 = nc.declare_dram_parameter("W0", [IN_DIM, D1], f32, isOutput=False)
    W1 = nc.declare_dram_parameter("W1", [D1, D1], f32, isOutput=False)
    W2 = nc.declare_dram_parameter("W2", [D1, D2], f32, isOutput=False)
    A0 = nc.declare_dram_parameter("A0", [D1, 8], f32, isOutput=False)
    A1 = nc.declare_dram_parameter("A1", [D1, 8], f32, isOutput=False)
    A2 = nc.declare_dram_parameter("A2", [D2, 8], f32, isOutput=False)
    out_ext = nc.declare_dram_parameter("out", [NLOC, NCLS], f32, isOutput=True)

    Ws = [W0, W1, W2]
    As = [A0, A1, A2]
    DL = [D1, D1, D2]
    ROWL = [ROW1, ROW1, ROW2]
    ROWPL = [ROWP1, ROWP1, ROWP2]
    GROW = DEG * ROWP1        # G tile width (elems), max over layers
    MSGW = DEG * D1

    with tile.TileContext(nc) as tc:
        with (
            tc.tile_pool(name="const", bufs=1) as cp,
            tc.tile_pool(name="resid", bufs=1) as rp,
            tc.tile_pool(name="wk", bufs=3) as wk,
            tc.tile_pool(name="seq", bufs=1) as sq,
            tc.tile_pool(name="gat", bufs=3) as gp,
            tc.tile_pool(name="psp", bufs=2, space="PSUM") as psp,
            tc.tile_pool(name="dram", bufs=1, space="DRAM") as dram,
        ):
            ident = cp.tile([P, P], f32)
            make_identity(nc, ident[:])

            wsb = []
            asb = []
            for l in range(3):
                wl, al = [], []
                for ic in range(2):
                    w = cp.tile([P, DL[l]], f32, name=f"w{l}_{ic}")
                    nc.sync.dma_start(out=w[:], in_=Ws[l][ic * P:(ic + 1) * P, :])
                    wl.append(w)
                nch = (DL[l] + P - 1) // P
                for ic in range(nch):
                    lo = ic * P
                    hi = min(DL[l], lo + P)
                    a = cp.tile([P, 8], f32, name=f"a{l}_{ic}")
                    nc.sync.dma_start(out=a[:hi - lo, :], in_=As[l][lo:hi, :])
                    al.append((a, hi - lo))
                wsb.append(wl)
                asb.append(al)

            h_res = rp.tile([P, TPC * D1], f32)          # 12.8 MB
            er_res = rp.tile([P, TPC * 4], bf16)
            idxs = rp.tile([P, TPC * DEG], mybir.dt.int32)
            nc.sync.dma_start(out=idxs[:], in_=idx_in[:])

            ag_in = [dram.tile([NLOC, ROWL[l]], bf16, name=f"agin{l}")
                     for l in range(3)]
            table = [dram.tile([NPAD, ROWL[l]], bf16, addr_space="Shared",
                               name=f"table{l}")
                     for l in range(3)]

            def dense_tile(l, t):
                """feat = h @ W for dst tile t of layer l; writes packed row
                slice to ag_in[l] and er to er_res."""
                DO = DL[l]
                ROW = ROWL[l]
                hT = []
                if l == 0:
                    for ic in range(2):
                        ht = wk.tile([P, P], f32, tag="ht", name=f"ht{l}_{t}_{ic}")
                        nc.sync.dma_start(
                            out=ht[:],
                            in_=hT0[t * IN_DIM + ic * P:
                                    t * IN_DIM + (ic + 1) * P, :])
                        hT.append(ht)
                else:
                    for ic in range(2):
                        tp = psp.tile([P, P], f32, tag="tp", bufs=2,
                                      name=f"tp{l}_{t}_{ic}")
                        nc.tensor.transpose(
                            tp[:],
                            h_res[:, t * D1 + ic * P: t * D1 + (ic + 1) * P],
                            ident[:])
                        ht = wk.tile([P, P], f32, tag="ht", name=f"ht{l}_{t}_{ic}")
                        nc.scalar.copy(ht[:], tp[:])
                        hT.append(ht)

                packed = wk.tile([P, ROW1], bf16, tag="pk", name=f"pk{l}_{t}")
                noc = (DO + P - 1) // P
                fTs = []
                # layer 0 runs before any gather work exists: Vector is idle,
                # so let it take half the PSUM->SBUF copies in the prologue
                cpy = nc.vector.tensor_copy if l == 0 else nc.scalar.copy
                for oc in range(noc):
                    lo = oc * P
                    hi = min(DO, lo + P)
                    w = hi - lo
                    fp = psp.tile([P, P], f32, tag="fp", bufs=2,
                                  name=f"fp{l}_{t}_{oc}")
                    nc.tensor.matmul(fp[:w, :], wsb[l][0][:, lo:hi], hT[0][:],
                                     start=True, stop=False)
                    nc.tensor.matmul(fp[:w, :], wsb[l][1][:, lo:hi], hT[1][:],
                                     start=False, stop=True)
                    fT = wk.tile([P, P], f32, tag="fT", name=f"fT{l}_{t}_{oc}")
                    cpy(fT[:w, :], fp[:w, :])
                    fTs.append((fT, w))
                    bk = psp.tile([P, P], f32, tag="bk", bufs=2,
                                  name=f"bk{l}_{t}_{oc}")
                    nc.tensor.transpose(bk[:, :w], fT[:w, :], ident[:w, :w])
                    nc.scalar.copy(packed[:, lo:hi], bk[:, :w])

                # el/er transposed directly: et2[n, 8] = fT^T @ A-chunk
                et2 = psp.tile([P, 8], f32, tag="et2", bufs=2, name=f"et2{l}_{t}")
                nf = len(fTs)
                for ic, (fT, w) in enumerate(fTs):
                    nc.tensor.matmul(et2[:, :], fTs[ic][0][:fTs[ic][1], :],
                                     asb[l][ic][0][:fTs[ic][1], :],
                                     start=(ic == 0), stop=(ic == nf - 1))
                nc.vector.tensor_copy(packed[:, DO:DO + 4], et2[:, 0:4])
                nc.vector.tensor_copy(er_res[:, t * 4:(t + 1) * 4], et2[:, 4:8])

                nc.sync.dma_start(out=ag_in[l][t * P:(t + 1) * P, :],
                                  in_=packed[:, :ROW])

            def share_chunk(l, j):
                """AllGather chunk j of layer l's table (rows j*CHROWS ..)."""
                r0 = j * CHROWS
                r1 = r0 + CHROWS
                nc.gpsimd.collective_compute(
                    "AllGather",
                    mybir.AluOpType.bypass,
                    replica_groups=[list(range(NCORES))],
                    ins=[ag_in[l][r0:r1]],
                    outs=[table[l][j * NCORES * CHROWS:(j + 1) * NCORES * CHROWS]],
                )

            def gather_agg_tile(l, t):
                DO = DL[l]
                ROW = ROWL[l]
                ROWP = ROWPL[l]
                hd = DO // HEADS
                G = gp.tile([P, GROW], bf16, tag="G", name=f"G{l}_{t}")
                Gv = G[:, :DEG * ROWP].rearrange("p (k r) -> p k r", k=DEG)
                # per-edge-slot indirect DMA ([128,1] offsets): the only
                # form the indirect1d ucode supports (one desc per partition)
                for k in range(DEG):
                    nc.gpsimd.indirect_dma_start(
                        out=Gv[:, k, :ROW],
                        out_offset=None,
                        in_=table[l][:],
                        in_offset=bass.IndirectOffsetOnAxis(
                            ap=idxs[:, t * DEG + k:t * DEG + k + 1], axis=0),
                    )
                # e = lrelu(el_src + er_dst) in (k h) layout
                e = wk.tile([P, 64], f32, tag="e", name=f"e{l}_{t}")
                el_view = Gv[:, :, DO:DO + 4]
                er_b = er_res[:, t * 4:(t + 1) * 4] \
                    .unsqueeze(1).broadcast_to([P, DEG, 4])
                nc.vector.tensor_tensor(
                    out=e[:].rearrange("p (k h) -> p k h", h=4),
                    in0=el_view, in1=er_b, op=mybir.AluOpType.add)
                esc = wk.tile([P, 64], f32, tag="esc", name=f"esc{l}_{t}")
                nc.vector.tensor_scalar_mul(esc[:], e[:], NEG)
                nc.vector.tensor_max(e[:], e[:], esc[:])
                ex = wk.tile([P, 64], bf16, tag="ex", name=f"ex{l}_{t}")
                nc.scalar.activation(ex[:], e[:],
                                     mybir.ActivationFunctionType.Exp)
                den = wk.tile([P, 4], f32, tag="den", name=f"den{l}_{t}")
                nc.vector.tensor_reduce(
                    out=den[:], in_=ex[:].rearrange("p (k h) -> p h k", h=4),
                    axis=mybir.AxisListType.X, op=mybir.AluOpType.add)
                rden = wk.tile([P, 4], f32, tag="rden", name=f"rden{l}_{t}")
                nc.vector.reciprocal(rden[:], den[:])
                if l == 2:
                    nc.vector.tensor_scalar_mul(rden[:], rden[:], 1.0 / HEADS)
                alp = wk.tile([P, 64], bf16, tag="alp", name=f"alp{l}_{t}")
                nc.vector.tensor_tensor(
                    out=alp[:].rearrange("p (k h) -> p k h", h=4),
                    in0=ex[:].rearrange("p (k h) -> p k h", h=4),
                    in1=rden[:].unsqueeze(1).broadcast_to([P, DEG, 4]),
                    op=mybir.AluOpType.mult)

                # msg = G_feat * alpha (bcast over d; innermost h) -> 2x mode
                msg = sq.tile([P, MSGW], bf16, tag="msg", name=f"msg{l}_{t}")
                gfeat = Gv[:, :, 0:DO].rearrange("p k (d h) -> p k d h", h=HEADS)
                alp_b = alp[:].rearrange("p (k h) -> p k h", h=4) \
                    .unsqueeze(2).broadcast_to([P, DEG, hd, HEADS])
                nc.vector.tensor_tensor(
                    out=msg[:, :DEG * DO].rearrange(
                        "p (k d h) -> p k d h", k=DEG, h=HEADS),
                    in0=gfeat, in1=alp_b, op=mybir.AluOpType.mult)

                cur = msg
                width = DEG * DO
                lvl = 0
                while width > DO:
                    width //= 2
                    nxt = sq.tile([P, width], bf16, tag=f"s{lvl}",
                                  name=f"s{l}_{t}_{lvl}")
                    nc.vector.tensor_add(nxt[:], cur[:, 0:width],
                                         cur[:, width:2 * width])
                    cur = nxt
                    lvl += 1

                if l < 2:
                    nc.scalar.copy(h_res[:, t * D1:(t + 1) * D1], cur[:, :DO])
                else:
                    lg = wk.tile([P, NCLS], f32, tag="lg", name=f"lg{l}_{t}")
                    nc.vector.tensor_reduce(
                        out=lg[:],
                        in_=cur[:, :DO].rearrange("p (c h) -> p c h", h=HEADS),
                        axis=mybir.AxisListType.X, op=mybir.AluOpType.add)
                    nc.sync.dma_start(out=out_ext[t * P:(t + 1) * P, :],
                                      in_=lg[:])

            # ---------------- pipelined schedule ----------------
            for t in range(TPC):
                dense_tile(0, t)
                if (t + 1) % TCH == 0:
                    share_chunk(0, t // TCH)
            for l in range(3):
                for t in range(TPC):
                    gather_agg_tile(l, t)
                    if l < 2:
                        dense_tile(l + 1, t)
                        if (t + 1) % TCH == 0:
                            share_chunk(l + 1, t // TCH)

    nc.compile()
    return nc


def _row_map():
    """src global node g -> chunk-major table row."""
    g = np.arange(NPAD, dtype=np.int64)
    c = g // NLOC
    r = g % NLOC
    j = r // CHROWS
    return (j * (NCORES * CHROWS) + c * CHROWS + (r % CHROWS)).astype(np.int32)


def prep_inputs(row_ptr, col_ind, inputs, W0, al0, ar0, W1, al1, ar1,
                W2, al2, ar2):
    col = np.asarray(col_ind, np.int32).reshape(N, DEG)
    col_pad = np.zeros((NPAD, DEG), np.int32)
    col_pad[:N] = col
    rowmap = _row_map()
    col_pad = rowmap[col_pad]                               # chunk-major rows
    x = np.asarray(inputs, np.float32)
    x_pad = np.zeros((NPAD, IN_DIM), np.float32)
    x_pad[:N] = x

    W0n = np.asarray(W0, np.float32)[:, PERM1]
    W1n = np.asarray(W1, np.float32)[PERM1][:, PERM1]
    W2n = np.asarray(W2, np.float32)[PERM1][:, PERM2]
    a0 = _pack_a(al0, ar0, D1, HID)[PERM1]
    a1 = _pack_a(al1, ar1, D1, HID)[PERM1]
    a2 = _pack_a(al2, ar2, D2, NCLS)[PERM2]

    in_maps = []
    for c in range(NCORES):
        lo = c * NLOC
        xT = x_pad[lo:lo + NLOC].T
        hT0c = np.ascontiguousarray(
            xT.reshape(IN_DIM, TPC, P).transpose(1, 0, 2)
              .reshape(TPC * IN_DIM, P))
        ic = col_pad[lo:lo + NLOC]
        ia = ic.reshape(TPC, P, DEG).transpose(1, 0, 2).reshape(P, TPC * DEG)
        in_maps.append({
            "hT0": hT0c,
            "idx": np.ascontiguousarray(ia),
            "W0": W0n,
            "W1": W1n,
            "W2": W2n,
            "A0": a0, "A1": a1, "A2": a2,
        })
    return in_maps


_NC_CACHE = {}


def kernel(**inputs):
    from concourse.bass_utils import run_bass_kernel_spmd

    if "nc" not in _NC_CACHE:
        _NC_CACHE["nc"] = build_program()
    nc = _NC_CACHE["nc"]

    in_maps = prep_inputs(**inputs)

    trace = bool(int(os.environ.get("BASS_GAT_TRACE", "0")))
    res = run_bass_kernel_spmd(nc, in_maps, list(range(NCORES)), trace=trace)
    _NC_CACHE["last_exec_ns"] = res.exec_time_ns

    out = np.concatenate([res.results[c]["out"] for c in range(NCORES)], axis=0)
    return np.ascontiguousarray(out[:N])
